# revision 1
# baseline (speedup 1.0000x reference)
"""Grouped-Query Attention (Gemma3-style, sliding-window) Trainium2 kernel.

Sharding: 8 cores = (batch b in {0,1}) x (kv-group G in {0..3}).
Each core computes, for its batch's tokens:
  - k/v projections for group G, q projections for heads {G, G+4}
    (the reference module's reshape pairs q-head h with kv-group h % 4),
  - qk-rmsnorm, sliding-window causal attention for its 2 heads,
  - partial output projection through the matching 512 rows of Wo.
Host sums the 4 partials per batch.

Layouts: host pre-transposes x to xT and pre-tiles everything so every DMA
is contiguous per partition. All matmuls in fp16 (full PE rate), softmax and
norm statistics in fp32.

Engine notes:
  - scores are computed transposed (S^T tiles [k,q]) so exp writes P^T
    straight to SBUF, ready as the P@V lhsT -- no PE transposes of P.
  - V tiles carry an extra ones column, so the P@V matmul also produces the
    softmax row sums for free (softmax skips max-subtraction; qk-rmsnorm
    bounds |s| <= 16 mathematically, ~5.7 actually).
  - rsqrt for rmsnorm is exp(-0.5*ln(x)) so every ACT op (Square, Ln, Exp,
    Copy) lives in one activation-function set: no table reloads.
  - scale 1/16 and (1+q_scale) are folded into one per-partition multiplier
    applied during the q transpose copy.
"""

import math
from contextlib import ExitStack

import numpy as np

import concourse.bass as bass
import concourse.tile as tile
from concourse import bacc, mybir
from concourse.bass import ts, ds
from concourse.bass_utils import run_bass_kernel_spmd
from concourse.masks import make_identity

F16 = mybir.dt.float16
F32 = mybir.dt.float32
AF = mybir.ActivationFunctionType
ALU = mybir.AluOpType
_MY_FUNCS = {AF.Exp, AF.Ln, AF.Copy, AF.Square}

# Steer Bacc's activation-table chooser so Square/Ln/Exp/Copy all resolve to
# the one function set that contains them all (natural_log_exp_and_others).
# Indices of the returned dict are preserved, so emitted act_func_set_ids stay
# valid; this only removes pointless table reloads between Ln and Exp.
import concourse.bacc as _bacc_mod
from concourse.hw_specs import get_activation_tables as _orig_gat

_ONE_SET = "natural_log_exp_and_others"


def _steered_gat(arch):
    tabs = _orig_gat(arch)
    if _ONE_SET not in tabs:
        return tabs
    return {name: (set(funcs) if name == _ONE_SET else set(funcs) - _MY_FUNCS)
            for name, funcs in tabs.items()}


_bacc_mod.get_activation_tables = _steered_gat

EPS = 1e-6
HD = 256  # head dim


def build_nc(T=2048, D=2560, WIN=1024):
    nT, nD, WT = T // 128, D // 128, WIN // 128
    nc = bacc.Bacc("TRN2", target_bir_lowering=False, debug=False)

    xt = nc.dram_tensor("xt", [nT, 128, nD, 128], F16, kind="ExternalInput").ap()
    wq = nc.dram_tensor("wq", [128, nD, 512], F16, kind="ExternalInput").ap()
    wkv = nc.dram_tensor("wkv", [128, nD, 512], F16, kind="ExternalInput").ap()
    wo = nc.dram_tensor("wo", [128, 4, D], F16, kind="ExternalInput").ap()
    qs = nc.dram_tensor("qs", [128, 2], F32, kind="ExternalInput").ap()
    ks = nc.dram_tensor("ks", [128, 2], F32, kind="ExternalInput").ap()
    mdiag = nc.dram_tensor("mdiag", [128, 128], F32, kind="ExternalInput").ap()
    medge = nc.dram_tensor("medge", [128, 128], F32, kind="ExternalInput").ap()
    outp = nc.dram_tensor("outp", [T, D], F32, kind="ExternalOutput").ap()

    with tile.TileContext(nc) as tc, ExitStack() as ctx:
        _body(ctx, tc, nT, nD, WT, D,
              xt, wq, wkv, wo, qs, ks, mdiag, medge, outp)

    nc.compile()
    return nc


def _body(ctx, tc, nT, nD, WT, D, xt, wq, wkv, wo, qs, ks, mdiag, medge, outp):
    nc = tc.nc

    const = ctx.enter_context(tc.tile_pool(name="const", bufs=1))
    acts = ctx.enter_context(tc.tile_pool(name="acts", bufs=1))
    work = ctx.enter_context(tc.tile_pool(name="work", bufs=3))
    nrm = ctx.enter_context(tc.tile_pool(name="nrm", bufs=2))
    ptp_pool = ctx.enter_context(tc.tile_pool(name="ptp", bufs=6))
    stats = ctx.enter_context(tc.tile_pool(name="stats", bufs=6))
    psum = ctx.enter_context(tc.tile_pool(name="psum", bufs=8, space="PSUM"))

    ident = const.tile([128, 128], F16, tag="ident")
    make_identity(nc, ident[:])
    bias_eps = const.tile([128, 1], F32, tag="bias_eps")
    nc.vector.memset(bias_eps[:], EPS)
    # tiny constants first (scalar/ACT dma queue)
    qs_sb = const.tile([128, 2], F32, tag="qs")
    nc.scalar.dma_start(qs_sb[:], qs)
    ks_sb = const.tile([128, 2], F32, tag="ks")
    nc.scalar.dma_start(ks_sb[:], ks)
    md_sb = const.tile([128, 128], F32, tag="md")
    nc.scalar.dma_start(md_sb[:], mdiag)
    me_sb = const.tile([128, 128], F32, tag="me")
    nc.scalar.dma_start(me_sb[:], medge)
    # weights as per-chunk tiles so the first projection matmul only waits
    # for its own chunk; interleaved q/kv emission order matches first use
    wq_c = [const.tile([128, 512], F16, tag=f"wq{c}", name=f"wq{c}")
            for c in range(nD)]
    wkv_c = [const.tile([128, 512], F16, tag=f"wkv{c}", name=f"wkv{c}")
             for c in range(nD)]
    wo_c = [const.tile([128, D], F16, tag=f"wo{c}", name=f"wo{c}")
            for c in range(4)]

    # full-length activations (single resident tiles)
    QT = acts.tile([128, 4, nT * 128], F16, tag="QT")    # [hd-chunk(h*2+c), t]
    KT = acts.tile([128, 2, nT * 128], F16, tag="KT")
    AOT = acts.tile([128, 4, nT * 128], F16, tag="AOT")
    V = [acts.tile([128, 257], F16, tag=f"v{j}", name=f"v{j}")
         for j in range(nT)]  # last column is ones (gives softmax row sums)

    state = {}
    xt_tiles = {}

    nD_a = nD // 2

    def xt_dma_emit(i):
        xt_a = work.tile([128, nD_a, 128], F16, tag="xta", name="xt_a")
        nc.sync.dma_start(xt_a[:], xt[i][:, 0:nD_a, :])
        xt_b = work.tile([128, nD - nD_a, 128], F16, tag="xtb", name="xt_b")
        nc.sync.dma_start(xt_b[:], xt[i][:, nD_a:nD, :])
        xt_tiles[i] = (xt_a, xt_b)

    def proj_emit(i):
        xt_a, xt_b = xt_tiles.pop(i)
        ps_q = psum.tile([128, 512], F32, tag="mm", name="ps_q")
        ps_kv = psum.tile([128, 512], F32, tag="mm", name="ps_kv")
        for c in range(nD):
            lt = xt_a[:, c, :] if c < nD_a else xt_b[:, c - nD_a, :]
            nc.tensor.matmul(ps_q[:], lhsT=lt, rhs=wq_c[c][:],
                             start=(c == 0), stop=(c == nD - 1))
            nc.tensor.matmul(ps_kv[:], lhsT=lt, rhs=wkv_c[c][:],
                             start=(c == 0), stop=(c == nD - 1))
        state[("ps", i)] = (ps_q, ps_kv)

    def norm_emit(i):
        ps_q, ps_kv = state.pop(("ps", i))
        nc.vector.tensor_copy(V[i][:, 0:256], ps_kv[:, 256:512])
        nc.vector.memset(V[i][:, 256:257], 1.0)
        # rmsnorm: rinv = exp(-0.5*ln(ssq/256 + eps)); q's extra 1/16 is
        # folded into the qs multiplier host-side
        sst = stats.tile([128, 3], F32, tag="sst", name="sst")
        for jj, src in enumerate((ps_q[:, 0:256], ps_q[:, 256:512],
                                  ps_kv[:, 0:256])):
            sq = nrm.tile([128, 256], F32, tag="sq", name="sq")
            nc.scalar.activation(sq[:], src, AF.Square,
                                 accum_out=sst[:, jj:jj + 1])
        lnv = stats.tile([128, 3], F32, tag="lnv", name="lnv")
        nc.scalar.activation(lnv[:], sst[:], AF.Ln, bias=bias_eps[:],
                             scale=1.0 / 256.0)
        rinv = stats.tile([128, 3], F32, tag="rinv", name="rinv")
        nc.scalar.activation(rinv[:], lnv[:], AF.Exp, scale=-0.5)
        qn = nrm.tile([128, 512], F16, tag="qn", name="qn")
        nc.vector.tensor_scalar_mul(qn[:, 0:256], ps_q[:, 0:256], rinv[:, 0:1])
        nc.vector.tensor_scalar_mul(qn[:, 256:512], ps_q[:, 256:512],
                                    rinv[:, 1:2])
        kn = nrm.tile([128, 256], F16, tag="kn", name="kn")
        nc.vector.tensor_scalar_mul(kn[:], ps_kv[:, 0:256], rinv[:, 2:3])
        state[("qn", i)] = (qn, kn)

    def transp_emit(i):
        qn, kn = state.pop(("qn", i))
        for cc in range(4):
            pt = psum.tile([128, 128], F16, tag="mm", name="pt_tr")
            nc.tensor.transpose(pt[:], qn[:, ts(cc, 128)], ident[:])
            nc.vector.tensor_scalar_mul(QT[:, cc, ts(i, 128)], pt[:],
                                        qs_sb[:, ds(cc % 2, 1)])
        for cc in range(2):
            pt = psum.tile([128, 128], F16, tag="mm", name="pt_tr")
            nc.tensor.transpose(pt[:], kn[:, ts(cc, 128)], ident[:])
            nc.vector.tensor_scalar_mul(KT[:, cc, ts(i, 128)], pt[:],
                                        ks_sb[:, ds(cc, 1)])

    def att_scores_pv_emit(i):
        jlo = max(0, i - WT)
        wlen = i - jlo + 1
        # both heads' scores+exp first, then both heads' P@V: the second
        # head's score matmuls hide the first head's exp latency on PE
        ptss = {}
        for h in range(2):
            # scores (transposed) + exp, in groups of 4 k-tiles per bank
            pts = []
            for g0 in range(0, wlen, 4):
                gn = min(4, wlen - g0)
                stg = psum.tile([128, 512], F32, tag="mm", name="stg")
                for s in range(gn):
                    jj = g0 + s
                    j = jlo + jj
                    for c in range(2):
                        nc.tensor.matmul(stg[:, ts(s, 128)],
                                         lhsT=KT[:, c, ts(j, 128)],
                                         rhs=QT[:, 2 * h + c, ts(i, 128)],
                                         start=(c == 0), stop=(c == 1))
                    if j == i:
                        nc.vector.tensor_add(stg[:, ts(s, 128)],
                                             stg[:, ts(s, 128)], md_sb[:])
                    elif i - j == WT:
                        nc.vector.tensor_add(stg[:, ts(s, 128)],
                                             stg[:, ts(s, 128)], me_sb[:])
                pt = ptp_pool.tile([128, 512], F16, tag="pt", name="pt_exp")
                nc.scalar.activation(pt[:, ds(0, gn * 128)],
                                     stg[:, ds(0, gn * 128)], AF.Exp)
                pts.append((pt, g0, gn))
            ptss[h] = pts
        for h in range(2):
            ps_o = psum.tile([128, 257], F32, tag="mm", name="ps_o")
            for pt, g0, gn in ptss[h]:
                for s in range(gn):
                    jj = g0 + s
                    nc.tensor.matmul(ps_o[:], lhsT=pt[:, ts(s, 128)],
                                     rhs=V[jlo + jj][:],
                                     start=(jj == 0), stop=(jj == wlen - 1))
            state[("po", i, h)] = ps_o

    def att_drain_emit(i, h):
        ps_o = state.pop(("po", i, h))
        rr = stats.tile([128, 1], F32, tag="rr", name="rr")
        nc.vector.reciprocal(rr[:], ps_o[:, 256:257])
        ao = nrm.tile([128, 256], F16, tag="ao", name="ao")
        nc.vector.tensor_scalar_mul(ao[:], ps_o[:, 0:256], rr[:])
        for c2 in range(2):
            pt = psum.tile([128, 128], F16, tag="mm", name="pt_tr")
            nc.tensor.transpose(pt[:], ao[:, ts(c2, 128)], ident[:])
            nc.vector.tensor_copy(AOT[:, 2 * h + c2, ts(i, 128)], pt[:])

    def drain_outproj_emit_a(i):
        # head-0 drain, then head-0's share of the first two output-column
        # chunks (fills PE while head-1's normalization resolves)
        att_drain_emit(i, 0)
        ob = work.tile([128, D], F32, tag="ob", name="ob")
        ps3s = []
        for n in range(2):
            ps3 = psum.tile([128, 512], F32, tag="mm", name="ps3")
            for c in range(2):
                nc.tensor.matmul(ps3[:], lhsT=AOT[:, c, ts(i, 128)],
                                 rhs=wo_c[c][:, ts(n, 512)],
                                 start=(c == 0), stop=False)
            ps3s.append(ps3)
        state[("op", i)] = (ob, ps3s)

    def drain_outproj_emit_b(i):
        ob, ps3s = state.pop(("op", i))
        att_drain_emit(i, 1)
        for n in range(2):
            for c in range(2, 4):
                nc.tensor.matmul(ps3s[n][:], lhsT=AOT[:, c, ts(i, 128)],
                                 rhs=wo_c[c][:, ts(n, 512)],
                                 start=False, stop=(c == 3))
            nc.scalar.activation(ob[:, ts(n, 512)], ps3s[n][:], AF.Copy)
        for n in range(2, D // 512):
            ps3 = psum.tile([128, 512], F32, tag="mm", name="ps3")
            for c in range(4):
                nc.tensor.matmul(ps3[:], lhsT=AOT[:, c, ts(i, 128)],
                                 rhs=wo_c[c][:, ts(n, 512)],
                                 start=(c == 0), stop=(c == 3))
            nc.scalar.activation(ob[:, ts(n, 512)], ps3[:], AF.Copy)
        nc.scalar.dma_start(outp[ts(i, 128), :], ob[:])

    # DMA priming: first x tile, then weight chunks in first-use order
    # (inputs ride the SP queue; weights/outputs ride the ACT queue).
    xt_dma_emit(0)
    for c in range(nD):
        nc.scalar.dma_start(wq_c[c][:], wq[:, c, :])
        nc.scalar.dma_start(wkv_c[c][:], wkv[:, c, :])
    xt_dma_emit(1)
    for c in range(4):
        nc.scalar.dma_start(wo_c[c][:], wo[:, c, :])

    # software-pipelined emission: iteration i's projections (long, dependency
    # free on PE) are emitted before iteration i-1's attention consumers so
    # every engine's in-order queue stays fed.
    for i in range(nT):
        proj_emit(i)
        if i + 2 < nT:
            xt_dma_emit(i + 2)
        if i > 0:
            transp_emit(i - 1)
            att_scores_pv_emit(i - 1)
            drain_outproj_emit_a(i - 1)
        norm_emit(i)
        if i > 0:
            drain_outproj_emit_b(i - 1)
    transp_emit(nT - 1)
    att_scores_pv_emit(nT - 1)
    drain_outproj_emit_a(nT - 1)
    drain_outproj_emit_b(nT - 1)


def make_core_inputs(x, Wq, Wk, Wv, Wo, q_scale, k_scale, T=2048, D=2560):
    """Per-core input dicts (host-side sharding + layout prep)."""
    nT, nD = T // 128, D // 128
    row = np.arange(128)[:, None]   # k index within S^T tile
    col = np.arange(128)[None, :]   # q index
    mdiag = np.where(row <= col, 0.0, -1e30).astype(np.float32)
    medge = np.where(row >= col + 1, 0.0, -1e30).astype(np.float32)
    qs = np.ascontiguousarray(
        ((1.0 + q_scale) / 16.0).astype(np.float32).reshape(2, 128).T)
    ks = np.ascontiguousarray((1.0 + k_scale).astype(np.float32).reshape(2, 128).T)

    in_maps = []
    for core in range(8):
        b, G = core // 4, core % 4
        h0, h1 = G, G + 4
        wq = np.concatenate(
            [Wq[:, 256 * h0:256 * (h0 + 1)], Wq[:, 256 * h1:256 * (h1 + 1)]], 1)
        wkv = np.concatenate(
            [Wk[:, 256 * G:256 * (G + 1)], Wv[:, 256 * G:256 * (G + 1)]], 1)
        wo = np.concatenate(
            [Wo[256 * h0:256 * (h0 + 1)], Wo[256 * h1:256 * (h1 + 1)]], 0)
        xT = x[b].T  # [D, T]
        xt = np.ascontiguousarray(
            xT.reshape(nD, 128, nT, 128).transpose(2, 1, 0, 3)).astype(np.float16)
        in_maps.append({
            "xt": xt,
            "wq": np.ascontiguousarray(
                wq.reshape(nD, 128, 512).transpose(1, 0, 2)).astype(np.float16),
            "wkv": np.ascontiguousarray(
                wkv.reshape(nD, 128, 512).transpose(1, 0, 2)).astype(np.float16),
            "wo": np.ascontiguousarray(
                wo.reshape(4, 128, D).transpose(1, 0, 2)).astype(np.float16),
            "qs": qs, "ks": ks, "mdiag": mdiag, "medge": medge,
        })
    return in_maps


_NC_CACHE = {}


def _get_nc(T=2048, D=2560, WIN=1024):
    key = (T, D, WIN)
    if key not in _NC_CACHE:
        _NC_CACHE[key] = build_nc(T, D, WIN)
    return _NC_CACHE[key]


def run_cores(inputs, trace=False):
    nc = _get_nc()
    in_maps = make_core_inputs(**inputs)
    res = run_bass_kernel_spmd(nc, in_maps, list(range(8)), trace=trace)
    B, T, D = inputs["x"].shape
    out = np.zeros((B, T, D), np.float32)
    for core in range(8):
        out[core // 4] += res.results[core]["outp"]
    return out, res


def kernel(x, Wq, Wk, Wv, Wo, q_scale, k_scale):
    out, _ = run_cores(dict(x=x, Wq=Wq, Wk=Wk, Wv=Wv, Wo=Wo,
                            q_scale=q_scale, k_scale=k_scale))
    return out

